# revision 2
# baseline (speedup 1.0000x reference)
"""Trainium2 Bass kernel for nn_DebugQuantizedLinear.

Computes out = x @ W_deq.T where
  W_deq = ((W_q - zeros) * scales).reshape(K, N) * mu2[:, None] * mu1[None, :]
  x: [B, N] f32, W_q: [K, N] int32 (values 0..15), out: [B, K] f32
  K=11008, N=4096, B=8192, group size 64 along N (NG=64 groups).

Strategy (8 NeuronCores, tensor-parallel along K):
  - K padded 11008 -> 11264 = 8 * 1408; core c owns rows [c*1408, (c+1)*1408).
  - Host supplies x transposed (xT [N, B] f32, replicated) so the contraction
    dim N lands on SBUF partitions for both matmul operands, plus W_q
    pre-transposed/packed as u8 (17*Q) so the dequantized W^T is produced
    directly in matmul-stationary layout by the Vector engine - the PE does
    matmuls only (the previous version burned ~80us of PE time on 128x128
    transposes and serialized phase 1 behind a single DMA queue).
  - Dequant (phase 1): per (kt-quad, n-tile) mini [128n, 512k]:
      wdqT = (17Q - z17) * (s*mu2) * (mu1/17)
    with z17 (u8) and s*mu2 (bf16) broadcast-DMA'd from compact per-group
    rows (partition_broadcast), as two DVE ops (tensor_tensor sub +
    scalar_tensor_tensor mul-mul with per-partition mu1/17).
    kt-quad-major order so h=0 matmuls consume kt 0..3 while kt 4..7
    are still being produced.
  - Phase 2: stream xT in 512-column half-panels (cast f32->fp16 by DMA),
    accumulate out^T tiles [128 k, 512 b] in PSUM over the 32 n-tiles,
    drain to SBUF, DMA to DRAM outT [1408, B] f32.
  - Host assembles out[B, K] from the 8 outT shards (transpose + concat).

DMA queue map (keeps the z/s broadcast writes off the critical queues):
  sync   (SP HWDGE):  wq minis + outT writes
  scalar (ACT HWDGE): z17 broadcasts (+ PSUM drains as engine compute)
  gpsimd (4x SWDGE):  x half-panels + s broadcasts
  vector:             dequant compute only

fp16 weights/activations with fp32 PSUM accumulation, z in u8 (1/17 step)
and s*mu2 in bf16 give ~4e-3 relative error vs the f32 reference.
"""

import os
from contextlib import ExitStack

import numpy as np

K, N, B = 11008, 4096, 8192
GROUP = 64
NG = N // GROUP
NCORES = 8
KC = 1408               # per-core padded K rows
KPAD = KC * NCORES      # 11264
P = 128
NKT = KC // P           # 11 k-tiles per core
NNT = N // P            # 32 n-tiles
KTQ_W = (512, 512, 384)  # kt-quad widths (kt 0-3, 4-7, 8-10)
KTQ_OF = (0, 512, 1024)

_PROGRAM_CACHE = {}
LAST_RESULTS = None     # BassKernelResults of the most recent run (for test.py)


def _build_program(b=B, bh=512):
    """Build the SPMD Bass program (identical on all cores)."""
    import concourse.bacc as bacc
    import concourse.mybir as mybir
    from concourse.tile import TileContext

    f32 = mybir.dt.float32
    f16 = mybir.dt.float16
    bf16 = mybir.dt.bfloat16
    u8 = mybir.dt.uint8

    nh = b // bh            # half-panels
    sub = mybir.AluOpType.subtract
    mul = mybir.AluOpType.mult

    nc = bacc.Bacc(num_swdge_queues=4)
    xT = nc.declare_dram_parameter("xT", [N, b], f32, isOutput=False)
    # 17*W_q, transposed + kt-quad-swizzled: mini (ktq, nt) is a contiguous
    # [128, W] block at rows [ktq*N + nt*128, ...) (wq_b holds ktq=2).
    wq_a = nc.declare_dram_parameter("wq_a", [2 * N, 512], u8, isOutput=False)
    wq_b = nc.declare_dram_parameter("wq_b", [N, 384], u8, isOutput=False)
    zt = nc.declare_dram_parameter("zt", [NG, KC], u8, isOutput=False)     # round(17*z)
    st = nc.declare_dram_parameter("st", [NG, KC], bf16, isOutput=False)   # s*mu2
    mu1 = nc.declare_dram_parameter("mu1", [P, NNT], f32, isOutput=False)  # mu1/17
    outT = nc.declare_dram_parameter("outT", [KC, b], f32, isOutput=True)

    with TileContext(nc) as tc, ExitStack() as ctx:
        const = ctx.enter_context(tc.tile_pool(name="const", bufs=1))
        mu1_t = const.tile([P, NNT], f32, name="mu1_t")
        nc.sync.dma_start(out=mu1_t[:, :], in_=mu1[:, :])

        # SBUF-resident transposed dequantized weights: per (nt, ktq) a
        # [128 n-partitions, W k-cols] fp16 tile (96 tiles, ~88KB/partition).
        wdqT = [[const.tile([P, w], f16, name=f"wdqT_{nt}_{q}")
                 for q, w in enumerate(KTQ_W)] for nt in range(NNT)]

        wqpool = ctx.enter_context(tc.tile_pool(name="wqpool", bufs=6))
        zpool = ctx.enter_context(tc.tile_pool(name="zpool", bufs=6))
        spool = ctx.enter_context(tc.tile_pool(name="spool", bufs=6))
        tpool = ctx.enter_context(tc.tile_pool(name="tpool", bufs=4))
        xpool = ctx.enter_context(tc.tile_pool(name="xpool", bufs=8))
        opsum = ctx.enter_context(tc.tile_pool(name="opsum", bufs=6, space="PSUM"))
        opool = ctx.enter_context(tc.tile_pool(name="opool", bufs=8))

        def dequant_mini(ktq, nt):
            w = KTQ_W[ktq]
            qm = wqpool.tile([P, w], u8, name="qm")
            if ktq < 2:
                nc.sync.dma_start(
                    out=qm[:, :], in_=wq_a[ktq * N + nt * P: ktq * N + (nt + 1) * P, :w])
            else:
                nc.sync.dma_start(
                    out=qm[:, :], in_=wq_b[nt * P:(nt + 1) * P, :w])
            ksl = slice(KTQ_OF[ktq], KTQ_OF[ktq] + w)
            zm = zpool.tile([P, w], u8, name="zm")
            sm = spool.tile([P, w], bf16, name="sm")
            for gi in range(2):
                g = 2 * nt + gi
                psl = slice(gi * GROUP, (gi + 1) * GROUP)
                nc.scalar.dma_start(
                    out=zm[psl, :], in_=zt[g, ksl].partition_broadcast(GROUP))
                nc.gpsimd.dma_start(
                    out=sm[psl, :], in_=st[g, ksl].partition_broadcast(GROUP))
            tm = tpool.tile([P, w], f16, name="tm")
            nc.vector.tensor_tensor(out=tm[:, :], in0=qm[:, :], in1=zm[:, :], op=sub)
            nc.vector.scalar_tensor_tensor(
                out=wdqT[nt][ktq][:, :], in0=tm[:, :],
                scalar=mu1_t[:, nt:nt + 1], in1=sm[:, :], op0=mul, op1=mul)

        def load_x_half(h):
            # 4 chunk tiles of 8 n-tiles each so early matmuls only wait on
            # the first chunk's DMA (f32->f16 cast in the DMA).
            src = xT[:, h * bh:(h + 1) * bh].rearrange("(t p) b -> p t b", p=P)
            chunks = []
            for c in range(4):
                xc = xpool.tile([P, 8, bh], f16, name="xc")
                nc.gpsimd.dma_start(out=xc[:, :, :], in_=src[:, c * 8:(c + 1) * 8, :])
                chunks.append(xc)
            return chunks

        def matmuls(h, kt, xh):
            ktq, j = divmod(kt, 4)
            ps = opsum.tile([P, bh], f32, name="ops")
            for nt in range(NNT):
                nc.tensor.matmul(
                    ps[:, :],
                    lhsT=wdqT[nt][ktq][:, j * P:(j + 1) * P],
                    rhs=xh[nt // 8][:, nt % 8, :],
                    start=(nt == 0), stop=(nt == NNT - 1))
            ot = opool.tile([P, bh], f32, name="ot")
            nc.scalar.copy(ot[:, :], ps[:, :])
            nc.sync.dma_start(
                out=outT[kt * P:(kt + 1) * P, h * bh:(h + 1) * bh], in_=ot[:, :])

        # h=0 rides along with dequant production, kt-quad by kt-quad.
        for nt in range(NNT):
            dequant_mini(0, nt)
        xh = load_x_half(0)
        for kt in range(4):
            matmuls(0, kt, xh)
        for nt in range(NNT):
            dequant_mini(1, nt)
        for kt in range(4, 8):
            matmuls(0, kt, xh)
        for nt in range(NNT):
            dequant_mini(2, nt)
        for kt in range(8, NKT):
            matmuls(0, kt, xh)
        for h in range(1, nh):
            xh = load_x_half(h)
            for kt in range(NKT):
                matmuls(h, kt, xh)

    nc.finalize()
    return nc


def _get_program(key=()):
    if key not in _PROGRAM_CACHE:
        _PROGRAM_CACHE[key] = _build_program(*key) if key else _build_program()
    return _PROGRAM_CACHE[key]


def kernel(x, W_q, zeros, scales, mu1, mu2):
    global LAST_RESULTS
    import ml_dtypes
    from concourse.bass_utils import run_bass_kernel_spmd

    x = np.asarray(x)
    W_q = np.asarray(W_q)
    zeros = np.asarray(zeros)
    scales = np.asarray(scales)
    mu1 = np.asarray(mu1)
    mu2 = np.asarray(mu2)

    # Host-side layout prep: transpose x; pack 17*W_q as u8, transposed and
    # kt-quad swizzled; z as round(17*z) u8; fold mu2 into s (bf16) and the
    # 1/17 into mu1.
    xT = np.ascontiguousarray(x.T)                      # [N, B] f32
    wq17 = np.zeros((KPAD, N), dtype=np.uint8)
    wq17[:K] = (W_q * 17).astype(np.uint8)
    z17 = np.zeros((KPAD, NG), dtype=np.uint8)
    z17[:K] = np.rint(zeros.reshape(K, NG) * 17.0).astype(np.uint8)
    s_f = np.zeros((KPAD, NG), dtype=np.float32)
    s_f[:K] = scales.reshape(K, NG) * mu2[:, None]
    mu1_r = np.ascontiguousarray(
        (mu1 / 17.0).astype(np.float32).reshape(NNT, P).T)  # [128, nnt]

    in_maps = []
    for c in range(NCORES):
        lo, hi = c * KC, (c + 1) * KC
        wqT_c = wq17[lo:hi].T                            # [N, KC] u8
        wq_q = wqT_c.reshape(N, NKT, P)
        wq_a = np.ascontiguousarray(
            np.concatenate([wq_q[:, 0:4].reshape(N, 512),
                            wq_q[:, 4:8].reshape(N, 512)], axis=0))
        wq_b = np.ascontiguousarray(wq_q[:, 8:11].reshape(N, 384))
        in_maps.append({
            "xT": xT,
            "wq_a": wq_a,
            "wq_b": wq_b,
            "zt": np.ascontiguousarray(z17[lo:hi].T),            # [NG, KC]
            "st": np.ascontiguousarray(
                s_f[lo:hi].T.astype(ml_dtypes.bfloat16)),        # [NG, KC]
            "mu1": mu1_r,
        })

    nc = _get_program()
    trace = bool(os.environ.get("KERNEL_TRACE"))
    res = run_bass_kernel_spmd(nc, in_maps, list(range(NCORES)), trace=trace)
    LAST_RESULTS = res

    out = np.empty((B, K), dtype=np.float32)
    for c in range(NCORES):
        lo = c * KC
        hi = min(lo + KC, K)
        out[:, lo:hi] = res.results[c]["outT"][:hi - lo].T
    return out


# revision 5
# speedup vs baseline: 1.0018x; 1.0018x over previous
"""Trainium2 Bass kernel for nn_DebugQuantizedLinear.

Computes out = x @ W_deq.T where
  W_deq = ((W_q - zeros) * scales).reshape(K, N) * mu2[:, None] * mu1[None, :]
  x: [B, N] f32, W_q: [K, N] int32 (values 0..15), out: [B, K] f32
  K=11008, N=4096, B=8192, group size 64 along N (NG=64 groups).

Strategy (8 NeuronCores, tensor-parallel along K):
  - K padded 11008 -> 11264 = 8 * 1408; core c owns rows [c*1408, (c+1)*1408).
  - Host supplies x transposed (xT [N, B] f32, replicated) so the contraction
    dim N lands on SBUF partitions for both matmul operands, plus W_q
    pre-transposed/packed as u8 (17*Q) so the dequantized W^T is produced
    directly in matmul-stationary layout by the Vector engine - the PE does
    matmuls only (the previous version burned ~80us of PE time on 128x128
    transposes and serialized phase 1 behind a single DMA queue).
  - Dequant (phase 1): per (kt-quad, n-tile) mini [128n, 512k]:
      wdqT = (17Q - z17) * (s*mu2) * (mu1/17)
    with z17 (u8) and s*mu2 (bf16) broadcast-DMA'd from compact per-group
    rows (partition_broadcast), as two DVE ops (tensor_tensor sub +
    scalar_tensor_tensor mul-mul with per-partition mu1/17).
    kt-quad-major order so h=0 matmuls consume kt 0..3 while kt 4..7
    are still being produced.
  - Phase 2: stream xT in 512-column half-panels (cast f32->fp16 by DMA),
    accumulate out^T tiles [128 k, 512 b] in PSUM over the 32 n-tiles,
    drain to SBUF, DMA to DRAM outT [1408, B] f32.
  - Host assembles out[B, K] from the 8 outT shards (transpose + concat).

DMA queue map (keeps the z/s broadcast writes off the critical queues):
  sync   (SP HWDGE):  wq minis + outT writes
  scalar (ACT HWDGE): z17 broadcasts (+ PSUM drains as engine compute)
  gpsimd (4x SWDGE):  x half-panels + s broadcasts
  vector:             dequant compute only

fp16 weights/activations with fp32 PSUM accumulation, z in u8 (1/17 step)
and s*mu2 in f16 (x32 scale shift, folded back via mu1) give ~3e-3
relative error vs the f32 reference.
"""

import os
from contextlib import ExitStack

import numpy as np

K, N, B = 11008, 4096, 8192
GROUP = 64
NG = N // GROUP
NCORES = 8
KC = 1408               # per-core padded K rows
KPAD = KC * NCORES      # 11264
P = 128
NKT = KC // P           # 11 k-tiles per core
NNT = N // P            # 32 n-tiles
KTQ_W = (512, 512, 384)  # kt-quad widths (kt 0-3, 4-7, 8-10)
KTQ_OF = (0, 512, 1024)

_PROGRAM_CACHE = {}
LAST_RESULTS = None     # BassKernelResults of the most recent run (for test.py)


def _build_program(b=B, bh=512):
    """Build the SPMD Bass program (identical on all cores)."""
    import concourse.bacc as bacc
    import concourse.mybir as mybir
    from concourse.tile import TileContext

    f32 = mybir.dt.float32
    f16 = mybir.dt.float16
    bf16 = mybir.dt.bfloat16
    u8 = mybir.dt.uint8

    nh = b // bh            # half-panels
    sub = mybir.AluOpType.subtract
    mul = mybir.AluOpType.mult

    nc = bacc.Bacc(num_swdge_queues=4)
    xT = nc.declare_dram_parameter("xT", [N, b], f32, isOutput=False)
    # 17*W_q, transposed + kt-quad-swizzled: mini (ktq, nt) is a contiguous
    # [128, W] block at rows [ktq*N + nt*128, ...) (wq_b holds ktq=2).
    wq_a = nc.declare_dram_parameter("wq_a", [2 * N, 512], u8, isOutput=False)
    wq_b = nc.declare_dram_parameter("wq_b", [N, 384], u8, isOutput=False)
    # even/odd group rows split so each broadcast DMA stays within the
    # 3-dim AP balance limit (z: round(17*z) u8; s: s*mu2*32 f16)
    zt_e = nc.declare_dram_parameter("zt_e", [NG // 2, KC], u8, isOutput=False)
    zt_o = nc.declare_dram_parameter("zt_o", [NG // 2, KC], u8, isOutput=False)
    st_e = nc.declare_dram_parameter("st_e", [NG // 2, KC], f16, isOutput=False)
    st_o = nc.declare_dram_parameter("st_o", [NG // 2, KC], f16, isOutput=False)
    mu1 = nc.declare_dram_parameter("mu1", [P, NNT], f32, isOutput=False)  # mu1/17
    outT = nc.declare_dram_parameter("outT", [KC, b], f32, isOutput=True)

    with TileContext(nc) as tc, ExitStack() as ctx:
        const = ctx.enter_context(tc.tile_pool(name="const", bufs=1))
        mu1_t = const.tile([P, NNT], f32, name="mu1_t")
        nc.sync.dma_start(out=mu1_t[:, :], in_=mu1[:, :])

        # SBUF-resident transposed dequantized weights: per (ktq, 4-nt chunk)
        # a [128 n-partitions, 4 nt, W k-cols] fp16 tile (~88KB/partition).
        NCH = NNT // 4
        wdqT = [[const.tile([P, 4, w], f16, name=f"wdqT_{q}_{c}")
                 for c in range(NCH)] for q, w in enumerate(KTQ_W)]

        wqpool = ctx.enter_context(tc.tile_pool(name="wqpool", bufs=3))
        zpool = ctx.enter_context(tc.tile_pool(name="zpool", bufs=3))
        spool = ctx.enter_context(tc.tile_pool(name="spool", bufs=3))
        tpool = ctx.enter_context(tc.tile_pool(name="tpool", bufs=2))
        xpool = ctx.enter_context(tc.tile_pool(name="xpool", bufs=8))
        opsum = ctx.enter_context(tc.tile_pool(name="opsum", bufs=6, space="PSUM"))
        opool = ctx.enter_context(tc.tile_pool(name="opool", bufs=6))

        def dequant_chunk(ktq, c):
            # 4 n-tiles (nt = 4c..4c+3) x one kt-quad, single DMA each for
            # wq/z/s (trigger cost dominates small DMAs) + one batched sub.
            w = KTQ_W[ktq]
            qm = wqpool.tile([P, 4, w], u8, name="qm")
            if ktq < 2:
                src = wq_a[ktq * N + 4 * c * P: ktq * N + 4 * (c + 1) * P, :w]
            else:
                src = wq_b[4 * c * P: 4 * (c + 1) * P, :w]
            nc.sync.dma_start(
                out=qm[:, :, :], in_=src.rearrange("(nt p) w -> p nt w", p=P))
            ksl = slice(KTQ_OF[ktq], KTQ_OF[ktq] + w)
            zm = zpool.tile([P, 4, w], u8, name="zm")
            sm = spool.tile([P, 4, w], f16, name="sm")
            nsl = slice(4 * c, 4 * (c + 1))
            nc.scalar.dma_start(
                out=zm[0:GROUP, :, :],
                in_=zt_e[nsl, ksl].partition_broadcast(GROUP))
            nc.scalar.dma_start(
                out=zm[GROUP:P, :, :],
                in_=zt_o[nsl, ksl].partition_broadcast(GROUP))
            nc.gpsimd.dma_start(
                out=sm[0:GROUP, :, :],
                in_=st_e[nsl, ksl].partition_broadcast(GROUP))
            nc.gpsimd.dma_start(
                out=sm[GROUP:P, :, :],
                in_=st_o[nsl, ksl].partition_broadcast(GROUP))
            tm = tpool.tile([P, 4, w], f16, name="tm")
            nc.vector.tensor_tensor(
                out=tm[:, :, :], in0=qm[:, :, :], in1=zm[:, :, :], op=sub)
            for i in range(4):
                # mu1 is a per-partition scalar only within one n-tile
                nc.vector.scalar_tensor_tensor(
                    out=wdqT[ktq][c][:, i, :], in0=tm[:, i, :],
                    scalar=mu1_t[:, 4 * c + i: 4 * c + i + 1],
                    in1=sm[:, i, :], op0=mul, op1=mul)

        def load_x_half(h):
            # 4 chunk tiles of 8 n-tiles each so early matmuls only wait on
            # the first chunk's DMA (f32->f16 cast in the DMA).
            src = xT[:, h * bh:(h + 1) * bh].rearrange("(t p) b -> p t b", p=P)
            chunks = []
            for c in range(4):
                xc = xpool.tile([P, 8, bh], f16, name="xc")
                nc.gpsimd.dma_start(out=xc[:, :, :], in_=src[:, c * 8:(c + 1) * 8, :])
                chunks.append(xc)
            return chunks

        def matmuls(h, kt, xh):
            ktq, j = divmod(kt, 4)
            ps = opsum.tile([P, bh], f32, name="ops")
            for nt in range(NNT):
                nc.tensor.matmul(
                    ps[:, :],
                    lhsT=wdqT[ktq][nt // 4][:, nt % 4, j * P:(j + 1) * P],
                    rhs=xh[nt // 8][:, nt % 8, :],
                    start=(nt == 0), stop=(nt == NNT - 1))
            ot = opool.tile([P, bh], f32, name="ot")
            nc.scalar.copy(ot[:, :], ps[:, :])
            nc.sync.dma_start(
                out=outT[kt * P:(kt + 1) * P, h * bh:(h + 1) * bh], in_=ot[:, :])

        # h=0 rides along with dequant production, kt-quad by kt-quad.
        # x first so its gpsimd DMAs aren't queued behind s broadcasts.
        xh = load_x_half(0)
        for c in range(NCH):
            dequant_chunk(0, c)
        for kt in range(4):
            matmuls(0, kt, xh)
        for c in range(NCH):
            dequant_chunk(1, c)
        for kt in range(4, 8):
            matmuls(0, kt, xh)
        for c in range(NCH):
            dequant_chunk(2, c)
        for kt in range(8, NKT):
            matmuls(0, kt, xh)
        for h in range(1, nh):
            xh = load_x_half(h)
            for kt in range(NKT):
                matmuls(h, kt, xh)

    nc.finalize()
    return nc


def _get_program(key=()):
    if key not in _PROGRAM_CACHE:
        _PROGRAM_CACHE[key] = _build_program(*key) if key else _build_program()
    return _PROGRAM_CACHE[key]


def kernel(x, W_q, zeros, scales, mu1, mu2):
    global LAST_RESULTS
    import ml_dtypes
    from concourse.bass_utils import run_bass_kernel_spmd

    x = np.asarray(x)
    W_q = np.asarray(W_q)
    zeros = np.asarray(zeros)
    scales = np.asarray(scales)
    mu1 = np.asarray(mu1)
    mu2 = np.asarray(mu2)

    # Host-side layout prep: transpose x; pack 17*W_q as u8, transposed and
    # kt-quad swizzled; z as round(17*z) u8; fold mu2 into s (bf16) and the
    # 1/17 into mu1.
    xT = np.ascontiguousarray(x.T)                      # [N, B] f32
    wq17 = np.zeros((KPAD, N), dtype=np.uint8)
    wq17[:K] = (W_q * 17).astype(np.uint8)
    z17 = np.zeros((KPAD, NG), dtype=np.uint8)
    z17[:K] = np.rint(zeros.reshape(K, NG) * 17.0).astype(np.uint8)
    s_f = np.zeros((KPAD, NG), dtype=np.float32)
    s_f[:K] = scales.reshape(K, NG) * mu2[:, None]
    mu1_r = np.ascontiguousarray(
        (mu1 / (17.0 * 32.0)).astype(np.float32).reshape(NNT, P).T)  # [128, nnt]

    in_maps = []
    for c in range(NCORES):
        lo, hi = c * KC, (c + 1) * KC
        wqT_c = wq17[lo:hi].T                            # [N, KC] u8
        wq_q = wqT_c.reshape(N, NKT, P)
        wq_a = np.ascontiguousarray(
            np.concatenate([wq_q[:, 0:4].reshape(N, 512),
                            wq_q[:, 4:8].reshape(N, 512)], axis=0))
        wq_b = np.ascontiguousarray(wq_q[:, 8:11].reshape(N, 384))
        in_maps.append({
            "xT": xT,
            "wq_a": wq_a,
            "wq_b": wq_b,
            "zt_e": np.ascontiguousarray(z17[lo:hi].T[0::2]),    # [NG/2, KC]
            "zt_o": np.ascontiguousarray(z17[lo:hi].T[1::2]),
            "st_e": np.ascontiguousarray(
                (s_f[lo:hi].T[0::2] * 32.0).astype(np.float16)),
            "st_o": np.ascontiguousarray(
                (s_f[lo:hi].T[1::2] * 32.0).astype(np.float16)),
            "mu1": mu1_r,
        })

    nc = _get_program()
    trace = bool(os.environ.get("KERNEL_TRACE"))
    res = run_bass_kernel_spmd(nc, in_maps, list(range(NCORES)), trace=trace)
    LAST_RESULTS = res

    out = np.empty((B, K), dtype=np.float32)
    for c in range(NCORES):
        lo = c * KC
        hi = min(lo + KC, K)
        out[:, lo:hi] = res.results[c]["outT"][:hi - lo].T
    return out


# revision 6
# speedup vs baseline: 1.0111x; 1.0092x over previous
"""Trainium2 Bass kernel for nn_DebugQuantizedLinear.

Computes out = x @ W_deq.T where
  W_deq = ((W_q - zeros) * scales).reshape(K, N) * mu2[:, None] * mu1[None, :]
  x: [B, N] f32, W_q: [K, N] int32 (values 0..15), out: [B, K] f32
  K=11008, N=4096, B=8192, group size 64 along N (NG=64 groups).

Strategy (8 NeuronCores, tensor-parallel along K):
  - K padded 11008 -> 11264 = 8 * 1408; core c owns rows [c*1408, (c+1)*1408).
  - Host supplies x transposed (xT [N, B] f32, replicated) so the contraction
    dim N lands on SBUF partitions for both matmul operands, plus W_q
    pre-transposed/packed as u8 (17*Q) so the dequantized W^T is produced
    directly in matmul-stationary layout by the Vector engine - the PE does
    matmuls only (the previous version burned ~80us of PE time on 128x128
    transposes and serialized phase 1 behind a single DMA queue).
  - Dequant (phase 1): per (kt-quad, n-tile) mini [128n, 512k]:
      wdqT = (17Q - z17) * (s*mu2) * (mu1/17)
    with z17 (u8) and s*mu2 (bf16) broadcast-DMA'd from compact per-group
    rows (partition_broadcast), as two DVE ops (tensor_tensor sub +
    scalar_tensor_tensor mul-mul with per-partition mu1/17).
    kt-quad-major order so h=0 matmuls consume kt 0..3 while kt 4..7
    are still being produced.
  - Phase 2: stream xT in 512-column half-panels (cast f32->fp16 by DMA),
    accumulate out^T tiles [128 k, 512 b] in PSUM over the 32 n-tiles,
    drain to SBUF, DMA to DRAM outT [1408, B] f32.
  - Host assembles out[B, K] from the 8 outT shards (transpose + concat).

DMA queue map (keeps the z/s broadcast writes off the critical queues;
SWDGE queues are FIFO so broadcasts must not sit behind 23us x panels):
  sync   (SP HWDGE):  wq minis + outT writes
  scalar (ACT HWDGE): z17 + s broadcasts (+ PSUM drains as engine compute)
  gpsimd (4x SWDGE):  x half-panels only (+ the dequant sub as compute)
  vector:             per-nt scale ops

fp16 weights/activations with fp32 PSUM accumulation, z in u8 (1/17 step)
and s*mu2 in f16 (x32 scale shift, folded back via mu1) give ~3e-3
relative error vs the f32 reference.
"""

import os
from contextlib import ExitStack

import numpy as np

K, N, B = 11008, 4096, 8192
GROUP = 64
NG = N // GROUP
NCORES = 8
KC = 1408               # per-core padded K rows
KPAD = KC * NCORES      # 11264
P = 128
NKT = KC // P           # 11 k-tiles per core
NNT = N // P            # 32 n-tiles
KTQ_W = (512, 512, 384)  # kt-quad widths (kt 0-3, 4-7, 8-10)
KTQ_OF = (0, 512, 1024)

_PROGRAM_CACHE = {}
LAST_RESULTS = None     # BassKernelResults of the most recent run (for test.py)


def _build_program(b=B, bh=512):
    """Build the SPMD Bass program (identical on all cores)."""
    import concourse.bacc as bacc
    import concourse.mybir as mybir
    from concourse.tile import TileContext

    f32 = mybir.dt.float32
    f16 = mybir.dt.float16
    bf16 = mybir.dt.bfloat16
    u8 = mybir.dt.uint8

    nh = b // bh            # half-panels
    sub = mybir.AluOpType.subtract
    mul = mybir.AluOpType.mult

    nc = bacc.Bacc(num_swdge_queues=4)
    xT = nc.declare_dram_parameter("xT", [N, b], f32, isOutput=False)
    # 17*W_q, transposed + kt-quad-swizzled: mini (ktq, nt) is a contiguous
    # [128, W] block at rows [ktq*N + nt*128, ...) (wq_b holds ktq=2).
    wq_a = nc.declare_dram_parameter("wq_a", [2 * N, 512], u8, isOutput=False)
    wq_b = nc.declare_dram_parameter("wq_b", [N, 384], u8, isOutput=False)
    # even/odd group rows split so each broadcast DMA stays within the
    # 3-dim AP balance limit (z: round(17*z) u8; s: s*mu2*32 f16)
    zt_e = nc.declare_dram_parameter("zt_e", [NG // 2, KC], u8, isOutput=False)
    zt_o = nc.declare_dram_parameter("zt_o", [NG // 2, KC], u8, isOutput=False)
    st_e = nc.declare_dram_parameter("st_e", [NG // 2, KC], f16, isOutput=False)
    st_o = nc.declare_dram_parameter("st_o", [NG // 2, KC], f16, isOutput=False)
    mu1 = nc.declare_dram_parameter("mu1", [P, NNT], f32, isOutput=False)  # mu1/17
    outT = nc.declare_dram_parameter("outT", [KC, b], f32, isOutput=True)

    with TileContext(nc) as tc, ExitStack() as ctx:
        const = ctx.enter_context(tc.tile_pool(name="const", bufs=1))
        mu1_t = const.tile([P, NNT], f32, name="mu1_t")
        nc.sync.dma_start(out=mu1_t[:, :], in_=mu1[:, :])

        # SBUF-resident transposed dequantized weights: per (ktq, 4-nt chunk)
        # a [128 n-partitions, 4 nt, W k-cols] fp16 tile (~88KB/partition).
        NCH = NNT // 4
        wdqT = [[const.tile([P, 4, w], f16, name=f"wdqT_{q}_{c}")
                 for c in range(NCH)] for q, w in enumerate(KTQ_W)]

        wqpool = ctx.enter_context(tc.tile_pool(name="wqpool", bufs=3))
        zpool = ctx.enter_context(tc.tile_pool(name="zpool", bufs=3))
        spool = ctx.enter_context(tc.tile_pool(name="spool", bufs=3))
        tpool = ctx.enter_context(tc.tile_pool(name="tpool", bufs=2))
        xpool = ctx.enter_context(tc.tile_pool(name="xpool", bufs=8))
        opsum = ctx.enter_context(tc.tile_pool(name="opsum", bufs=6, space="PSUM"))
        opool = ctx.enter_context(tc.tile_pool(name="opool", bufs=6))

        def dequant_chunk(ktq, c):
            # 4 n-tiles (nt = 4c..4c+3) x one kt-quad, single DMA each for
            # wq/z/s (trigger cost dominates small DMAs) + one batched sub.
            w = KTQ_W[ktq]
            qm = wqpool.tile([P, 4, w], u8, name="qm")
            if ktq < 2:
                src = wq_a[ktq * N + 4 * c * P: ktq * N + 4 * (c + 1) * P, :w]
            else:
                src = wq_b[4 * c * P: 4 * (c + 1) * P, :w]
            nc.sync.dma_start(
                out=qm[:, :, :], in_=src.rearrange("(nt p) w -> p nt w", p=P))
            ksl = slice(KTQ_OF[ktq], KTQ_OF[ktq] + w)
            zm = zpool.tile([P, 4, w], u8, name="zm")
            sm = spool.tile([P, 4, w], f16, name="sm")
            nsl = slice(4 * c, 4 * (c + 1))
            nc.scalar.dma_start(
                out=zm[0:GROUP, :, :],
                in_=zt_e[nsl, ksl].partition_broadcast(GROUP))
            nc.scalar.dma_start(
                out=zm[GROUP:P, :, :],
                in_=zt_o[nsl, ksl].partition_broadcast(GROUP))
            nc.scalar.dma_start(
                out=sm[0:GROUP, :, :],
                in_=st_e[nsl, ksl].partition_broadcast(GROUP))
            nc.scalar.dma_start(
                out=sm[GROUP:P, :, :],
                in_=st_o[nsl, ksl].partition_broadcast(GROUP))
            tm = tpool.tile([P, 4, w], f16, name="tm")
            # sub on GpSimd: it is idle during h=0 while the DVE runs the
            # 4 per-nt scale ops; splitting keeps phase-1 production ahead
            # of the h=0 matmul sweep.
            nc.gpsimd.tensor_tensor(
                out=tm[:, :, :], in0=qm[:, :, :], in1=zm[:, :, :], op=sub)
            for i in range(4):
                # mu1 is a per-partition scalar only within one n-tile
                nc.vector.scalar_tensor_tensor(
                    out=wdqT[ktq][c][:, i, :], in0=tm[:, i, :],
                    scalar=mu1_t[:, 4 * c + i: 4 * c + i + 1],
                    in1=sm[:, i, :], op0=mul, op1=mul)

        def load_x_half(h):
            # 4 chunk tiles of 8 n-tiles each so early matmuls only wait on
            # the first chunk's DMA (f32->f16 cast in the DMA).
            src = xT[:, h * bh:(h + 1) * bh].rearrange("(t p) b -> p t b", p=P)
            chunks = []
            for c in range(4):
                xc = xpool.tile([P, 8, bh], f16, name="xc")
                nc.gpsimd.dma_start(out=xc[:, :, :], in_=src[:, c * 8:(c + 1) * 8, :])
                chunks.append(xc)
            return chunks

        def matmuls(h, kt, xh):
            ktq, j = divmod(kt, 4)
            ps = opsum.tile([P, bh], f32, name="ops")
            for nt in range(NNT):
                nc.tensor.matmul(
                    ps[:, :],
                    lhsT=wdqT[ktq][nt // 4][:, nt % 4, j * P:(j + 1) * P],
                    rhs=xh[nt // 8][:, nt % 8, :],
                    start=(nt == 0), stop=(nt == NNT - 1))
            ot = opool.tile([P, bh], f32, name="ot")
            nc.scalar.copy(ot[:, :], ps[:, :])
            nc.sync.dma_start(
                out=outT[kt * P:(kt + 1) * P, h * bh:(h + 1) * bh], in_=ot[:, :])

        # h=0 rides along with dequant production, kt-quad by kt-quad.
        # x first so its gpsimd DMAs aren't queued behind s broadcasts.
        xh = load_x_half(0)
        for c in range(NCH):
            dequant_chunk(0, c)
        for kt in range(4):
            matmuls(0, kt, xh)
        for c in range(NCH):
            dequant_chunk(1, c)
        for kt in range(4, 8):
            matmuls(0, kt, xh)
        for c in range(NCH):
            dequant_chunk(2, c)
        for kt in range(8, NKT):
            matmuls(0, kt, xh)
        for h in range(1, nh):
            xh = load_x_half(h)
            for kt in range(NKT):
                matmuls(h, kt, xh)

    nc.finalize()
    return nc


def _get_program(key=()):
    if key not in _PROGRAM_CACHE:
        _PROGRAM_CACHE[key] = _build_program(*key) if key else _build_program()
    return _PROGRAM_CACHE[key]


def kernel(x, W_q, zeros, scales, mu1, mu2):
    global LAST_RESULTS
    import ml_dtypes
    from concourse.bass_utils import run_bass_kernel_spmd

    x = np.asarray(x)
    W_q = np.asarray(W_q)
    zeros = np.asarray(zeros)
    scales = np.asarray(scales)
    mu1 = np.asarray(mu1)
    mu2 = np.asarray(mu2)

    # Host-side layout prep: transpose x; pack 17*W_q as u8, transposed and
    # kt-quad swizzled; z as round(17*z) u8; fold mu2 into s (bf16) and the
    # 1/17 into mu1.
    xT = np.ascontiguousarray(x.T)                      # [N, B] f32
    wq17 = np.zeros((KPAD, N), dtype=np.uint8)
    wq17[:K] = (W_q * 17).astype(np.uint8)
    z17 = np.zeros((KPAD, NG), dtype=np.uint8)
    z17[:K] = np.rint(zeros.reshape(K, NG) * 17.0).astype(np.uint8)
    s_f = np.zeros((KPAD, NG), dtype=np.float32)
    s_f[:K] = scales.reshape(K, NG) * mu2[:, None]
    mu1_r = np.ascontiguousarray(
        (mu1 / (17.0 * 32.0)).astype(np.float32).reshape(NNT, P).T)  # [128, nnt]

    in_maps = []
    for c in range(NCORES):
        lo, hi = c * KC, (c + 1) * KC
        wqT_c = wq17[lo:hi].T                            # [N, KC] u8
        wq_q = wqT_c.reshape(N, NKT, P)
        wq_a = np.ascontiguousarray(
            np.concatenate([wq_q[:, 0:4].reshape(N, 512),
                            wq_q[:, 4:8].reshape(N, 512)], axis=0))
        wq_b = np.ascontiguousarray(wq_q[:, 8:11].reshape(N, 384))
        in_maps.append({
            "xT": xT,
            "wq_a": wq_a,
            "wq_b": wq_b,
            "zt_e": np.ascontiguousarray(z17[lo:hi].T[0::2]),    # [NG/2, KC]
            "zt_o": np.ascontiguousarray(z17[lo:hi].T[1::2]),
            "st_e": np.ascontiguousarray(
                (s_f[lo:hi].T[0::2] * 32.0).astype(np.float16)),
            "st_o": np.ascontiguousarray(
                (s_f[lo:hi].T[1::2] * 32.0).astype(np.float16)),
            "mu1": mu1_r,
        })

    nc = _get_program()
    trace = bool(os.environ.get("KERNEL_TRACE"))
    res = run_bass_kernel_spmd(nc, in_maps, list(range(NCORES)), trace=trace)
    LAST_RESULTS = res

    out = np.empty((B, K), dtype=np.float32)
    for c in range(NCORES):
        lo = c * KC
        hi = min(lo + KC, K)
        out[:, lo:hi] = res.results[c]["outT"][:hi - lo].T
    return out


# revision 7
# speedup vs baseline: 1.0111x; 1.0000x over previous
"""Trainium2 Bass kernel for nn_DebugQuantizedLinear.

Computes out = x @ W_deq.T where
  W_deq = ((W_q - zeros) * scales).reshape(K, N) * mu2[:, None] * mu1[None, :]
  x: [B, N] f32, W_q: [K, N] int32 (values 0..15), out: [B, K] f32
  K=11008, N=4096, B=8192, group size 64 along N (NG=64 groups).

Strategy (8 NeuronCores, tensor-parallel along K):
  - K padded 11008 -> 11264 = 8 * 1408; core c owns rows [c*1408, (c+1)*1408).
  - Host supplies x transposed (xT [N, B] f32, replicated) so the contraction
    dim N lands on SBUF partitions for both matmul operands, plus W_q
    pre-transposed/packed as u8 (17*Q) so the dequantized W^T is produced
    directly in matmul-stationary layout by the Vector engine - the PE does
    matmuls only (the previous version burned ~80us of PE time on 128x128
    transposes and serialized phase 1 behind a single DMA queue).
  - Dequant (phase 1): per (kt-quad, n-tile) mini [128n, 512k]:
      wdqT = (17Q - z17) * (s*mu2) * (mu1/17)
    with z17 (u8) and s*mu2 (bf16) broadcast-DMA'd from compact per-group
    rows (partition_broadcast), as two DVE ops (tensor_tensor sub +
    scalar_tensor_tensor mul-mul with per-partition mu1/17).
    kt-quad-major order so h=0 matmuls consume kt 0..3 while kt 4..7
    are still being produced.
  - Phase 2: stream xT in 512-column half-panels (cast f32->fp16 by DMA),
    accumulate out^T tiles [128 k, 512 b] in PSUM over the 32 n-tiles,
    drain to SBUF, DMA to DRAM outT [1408, B] f32.
  - Host assembles out[B, K] from the 8 outT shards (transpose + concat).

DMA queue map (keeps the z/s broadcast writes off the critical queues;
SWDGE queues are FIFO so broadcasts must not sit behind 23us x panels):
  sync   (SP HWDGE):  wq minis + outT writes
  scalar (ACT HWDGE): z17 + s broadcasts (+ PSUM drains as engine compute)
  gpsimd (4x SWDGE):  x half-panels only (+ the dequant sub as compute)
  vector:             per-nt scale ops

fp16 weights/activations with fp32 PSUM accumulation, z in u8 (1/17 step)
and s*mu2 in f16 (x32 scale shift, folded back via mu1) give ~3e-3
relative error vs the f32 reference.
"""

import os
from contextlib import ExitStack

import numpy as np

K, N, B = 11008, 4096, 8192
GROUP = 64
NG = N // GROUP
NCORES = 8
KC = 1408               # per-core padded K rows
KPAD = KC * NCORES      # 11264
P = 128
NKT = KC // P           # 11 k-tiles per core
NNT = N // P            # 32 n-tiles
KTQ_W = (512, 512, 384)  # kt-quad widths (kt 0-3, 4-7, 8-10)
KTQ_OF = (0, 512, 1024)

_PROGRAM_CACHE = {}
LAST_RESULTS = None     # BassKernelResults of the most recent run (for test.py)


def _build_program(b=B, bh=512):
    """Build the SPMD Bass program (identical on all cores)."""
    import concourse.bacc as bacc
    import concourse.mybir as mybir
    from concourse.tile import TileContext

    f32 = mybir.dt.float32
    f16 = mybir.dt.float16
    bf16 = mybir.dt.bfloat16
    u8 = mybir.dt.uint8

    nh = b // bh            # half-panels
    sub = mybir.AluOpType.subtract
    mul = mybir.AluOpType.mult

    nc = bacc.Bacc(num_swdge_queues=4)
    xT = nc.declare_dram_parameter("xT", [N, b], f32, isOutput=False)
    # 17*W_q, transposed + kt-quad-swizzled: mini (ktq, nt) is a contiguous
    # [128, W] block at rows [ktq*N + nt*128, ...) (wq_b holds ktq=2).
    wq_a = nc.declare_dram_parameter("wq_a", [2 * N, 512], u8, isOutput=False)
    wq_b = nc.declare_dram_parameter("wq_b", [N, 384], u8, isOutput=False)
    # even/odd group rows split so each broadcast DMA stays within the
    # 3-dim AP balance limit (z: round(17*z) u8; s: s*mu2*32 f16)
    zt_e = nc.declare_dram_parameter("zt_e", [NG // 2, KC], u8, isOutput=False)
    zt_o = nc.declare_dram_parameter("zt_o", [NG // 2, KC], u8, isOutput=False)
    st_e = nc.declare_dram_parameter("st_e", [NG // 2, KC], f16, isOutput=False)
    st_o = nc.declare_dram_parameter("st_o", [NG // 2, KC], f16, isOutput=False)
    mu1 = nc.declare_dram_parameter("mu1", [P, NNT], f32, isOutput=False)  # mu1/17
    outT = nc.declare_dram_parameter("outT", [KC, b], f32, isOutput=True)

    with TileContext(nc) as tc, ExitStack() as ctx:
        const = ctx.enter_context(tc.tile_pool(name="const", bufs=1))
        mu1_t = const.tile([P, NNT], f32, name="mu1_t")
        nc.sync.dma_start(out=mu1_t[:, :], in_=mu1[:, :])

        # SBUF-resident transposed dequantized weights: per (ktq, 4-nt chunk)
        # a [128 n-partitions, 4 nt, W k-cols] fp16 tile (~88KB/partition).
        NCH = NNT // 4
        wdqT = [[const.tile([P, 4, w], f16, name=f"wdqT_{q}_{c}")
                 for c in range(NCH)] for q, w in enumerate(KTQ_W)]

        wqpool = ctx.enter_context(tc.tile_pool(name="wqpool", bufs=3))
        zpool = ctx.enter_context(tc.tile_pool(name="zpool", bufs=3))
        spool = ctx.enter_context(tc.tile_pool(name="spool", bufs=3))
        tpool = ctx.enter_context(tc.tile_pool(name="tpool", bufs=2))
        xpool = ctx.enter_context(tc.tile_pool(name="xpool", bufs=8))
        opsum = ctx.enter_context(tc.tile_pool(name="opsum", bufs=6, space="PSUM"))
        opool = ctx.enter_context(tc.tile_pool(name="opool", bufs=6))

        def dequant_chunk(ktq, c):
            # 4 n-tiles (nt = 4c..4c+3) x one kt-quad, single DMA each for
            # wq/z/s (trigger cost dominates small DMAs) + one batched sub.
            w = KTQ_W[ktq]
            qm = wqpool.tile([P, 4, w], u8, name="qm")
            if ktq < 2:
                src = wq_a[ktq * N + 4 * c * P: ktq * N + 4 * (c + 1) * P, :w]
            else:
                src = wq_b[4 * c * P: 4 * (c + 1) * P, :w]
            nc.sync.dma_start(
                out=qm[:, :, :], in_=src.rearrange("(nt p) w -> p nt w", p=P))
            ksl = slice(KTQ_OF[ktq], KTQ_OF[ktq] + w)
            zm = zpool.tile([P, 4, w], u8, name="zm")
            sm = spool.tile([P, 4, w], f16, name="sm")
            nsl = slice(4 * c, 4 * (c + 1))
            nc.scalar.dma_start(
                out=zm[0:GROUP, :, :],
                in_=zt_e[nsl, ksl].partition_broadcast(GROUP))
            nc.scalar.dma_start(
                out=zm[GROUP:P, :, :],
                in_=zt_o[nsl, ksl].partition_broadcast(GROUP))
            nc.scalar.dma_start(
                out=sm[0:GROUP, :, :],
                in_=st_e[nsl, ksl].partition_broadcast(GROUP))
            nc.scalar.dma_start(
                out=sm[GROUP:P, :, :],
                in_=st_o[nsl, ksl].partition_broadcast(GROUP))
            tm = tpool.tile([P, 4, w], f16, name="tm")
            nc.vector.tensor_tensor(
                out=tm[:, :, :], in0=qm[:, :, :], in1=zm[:, :, :], op=sub)
            for i in range(4):
                # mu1 is a per-partition scalar only within one n-tile
                nc.vector.scalar_tensor_tensor(
                    out=wdqT[ktq][c][:, i, :], in0=tm[:, i, :],
                    scalar=mu1_t[:, 4 * c + i: 4 * c + i + 1],
                    in1=sm[:, i, :], op0=mul, op1=mul)

        def load_x_half(h):
            # 4 chunk tiles of 8 n-tiles each so early matmuls only wait on
            # the first chunk's DMA (f32->f16 cast in the DMA).
            src = xT[:, h * bh:(h + 1) * bh].rearrange("(t p) b -> p t b", p=P)
            chunks = []
            for c in range(4):
                xc = xpool.tile([P, 8, bh], f16, name="xc")
                nc.gpsimd.dma_start(out=xc[:, :, :], in_=src[:, c * 8:(c + 1) * 8, :])
                chunks.append(xc)
            return chunks

        def matmuls(h, kt, xh):
            ktq, j = divmod(kt, 4)
            ps = opsum.tile([P, bh], f32, name="ops")
            for nt in range(NNT):
                nc.tensor.matmul(
                    ps[:, :],
                    lhsT=wdqT[ktq][nt // 4][:, nt % 4, j * P:(j + 1) * P],
                    rhs=xh[nt // 8][:, nt % 8, :],
                    start=(nt == 0), stop=(nt == NNT - 1))
            ot = opool.tile([P, bh], f32, name="ot")
            nc.scalar.copy(ot[:, :], ps[:, :])
            nc.sync.dma_start(
                out=outT[kt * P:(kt + 1) * P, h * bh:(h + 1) * bh], in_=ot[:, :])

        # h=0 AND h=1 ride along with dequant production, kt-quad by
        # kt-quad: two panels' worth of matmuls per produced kt-quad keeps
        # the PE ahead of the DVE dequant rate.
        xh0 = load_x_half(0)
        xh1 = load_x_half(1)
        for ktq, (lo, hi) in enumerate([(0, 4), (4, 8), (8, NKT)]):
            for c in range(NCH):
                dequant_chunk(ktq, c)
            for kt in range(lo, hi):
                matmuls(0, kt, xh0)
                matmuls(1, kt, xh1)
        for h in range(2, nh):
            xh = load_x_half(h)
            for kt in range(NKT):
                matmuls(h, kt, xh)

    nc.finalize()
    return nc


def _get_program(key=()):
    if key not in _PROGRAM_CACHE:
        _PROGRAM_CACHE[key] = _build_program(*key) if key else _build_program()
    return _PROGRAM_CACHE[key]


def kernel(x, W_q, zeros, scales, mu1, mu2):
    global LAST_RESULTS
    import ml_dtypes
    from concourse.bass_utils import run_bass_kernel_spmd

    x = np.asarray(x)
    W_q = np.asarray(W_q)
    zeros = np.asarray(zeros)
    scales = np.asarray(scales)
    mu1 = np.asarray(mu1)
    mu2 = np.asarray(mu2)

    # Host-side layout prep: transpose x; pack 17*W_q as u8, transposed and
    # kt-quad swizzled; z as round(17*z) u8; fold mu2 into s (bf16) and the
    # 1/17 into mu1.
    xT = np.ascontiguousarray(x.T)                      # [N, B] f32
    wq17 = np.zeros((KPAD, N), dtype=np.uint8)
    wq17[:K] = (W_q * 17).astype(np.uint8)
    z17 = np.zeros((KPAD, NG), dtype=np.uint8)
    z17[:K] = np.rint(zeros.reshape(K, NG) * 17.0).astype(np.uint8)
    s_f = np.zeros((KPAD, NG), dtype=np.float32)
    s_f[:K] = scales.reshape(K, NG) * mu2[:, None]
    mu1_r = np.ascontiguousarray(
        (mu1 / (17.0 * 32.0)).astype(np.float32).reshape(NNT, P).T)  # [128, nnt]

    in_maps = []
    for c in range(NCORES):
        lo, hi = c * KC, (c + 1) * KC
        wqT_c = wq17[lo:hi].T                            # [N, KC] u8
        wq_q = wqT_c.reshape(N, NKT, P)
        wq_a = np.ascontiguousarray(
            np.concatenate([wq_q[:, 0:4].reshape(N, 512),
                            wq_q[:, 4:8].reshape(N, 512)], axis=0))
        wq_b = np.ascontiguousarray(wq_q[:, 8:11].reshape(N, 384))
        in_maps.append({
            "xT": xT,
            "wq_a": wq_a,
            "wq_b": wq_b,
            "zt_e": np.ascontiguousarray(z17[lo:hi].T[0::2]),    # [NG/2, KC]
            "zt_o": np.ascontiguousarray(z17[lo:hi].T[1::2]),
            "st_e": np.ascontiguousarray(
                (s_f[lo:hi].T[0::2] * 32.0).astype(np.float16)),
            "st_o": np.ascontiguousarray(
                (s_f[lo:hi].T[1::2] * 32.0).astype(np.float16)),
            "mu1": mu1_r,
        })

    nc = _get_program()
    trace = bool(os.environ.get("KERNEL_TRACE"))
    res = run_bass_kernel_spmd(nc, in_maps, list(range(NCORES)), trace=trace)
    LAST_RESULTS = res

    out = np.empty((B, K), dtype=np.float32)
    for c in range(NCORES):
        lo = c * KC
        hi = min(lo + KC, K)
        out[:, lo:hi] = res.results[c]["outT"][:hi - lo].T
    return out
